# revision 11
# baseline (speedup 1.0000x reference)
"""EmergentSpinGlass fused kernel for 8 Trainium2 NeuronCores.

Reference computation (per batch b):
    s   = x @ W_spin.T + b_spin                       (N, D)
    mf  = mean_n s                                    (D,)
    g   = W_global @ mf                               (D,)   [same for all rows]
    EF  = s @ W_J.T                                   (N, D)
    A   = softmax(EF @ s.T / sqrt(D), axis=-1)        (N, N)
    LF  = A @ s                                       (N, D)
    out = tanh(beta * (s + g + LF))                   (N, D)

Sharding: 8 cores = 4 batches x 2 query-halves. Each core receives x^T for
its batch with its query half's rows permuted first (attention is
permutation-invariant over keys), computes s for all 2048 keys, and runs
the attention block for its 1024 queries.

Precision strategy (tolerance is 2e-2; measured ~2e-3):
  - Phase 1 (s = W_spin x + b) runs in float32r: the "+s" term dominates
    the output, so s is the accuracy anchor (kept in f32r for the final
    add via SQ and for the SN transposes).
  - Everything downstream of s runs in fp8 e4m3 with DoubleRow matmuls
    (2 MACs/cell/cycle): EF = (8 W_J) s, scores^T = STQ.T EF,
    LF = P.T (s+g).  W_J/W_global are pre-scaled x8 on the host so their
    fp8 encodings stay in the normal range; the /8 is folded into the
    softmax exp scale (and /256 into the g copy).
  - Softmax skips the running-max subtraction (|logits| < ~2 by
    construction) and is normalized AFTER the LF matmul: exp() outputs
    unnormalized P^T in fp8, row sums come from a tiny DoubleRow matmul
    against a ones vector (so normalization is exact for the quantized
    affinities), and LF is scaled by 1/rowsum on the DVE at the end.
  - g rides inside SN: since the affinities sum to 1 (after the rinv
    scale), storing SN = s + g makes the LF matmul emit LF + g directly.

Structure (all phases are dense back-to-back PE work; HAM stays warm):
  1. ph1: s^T tiles via f32r matmuls, streamed x^T chunks (double-
     buffered); DVE writes ST (f32r, +bias, mean-field accum), ACT writes
     STQ (fp8, +bias) from the same PSUM.
  2. EF: DoubleRow fp8 matmuls over the query half of STQ.
  3. g: DoubleRow rank-2 matmuls (mf8 stationary) + rank-1 broadcast to
     a full [128, D] gfull tile.
  4. Transposes: PE-transposes ST into key-partition layout; DVE adds
     gfull and writes SN (fp8), ACT copies the query half to SQ (f32r).
  5. scores^T per key tile: 8 DoubleRow matmuls + ACT exp -> PT fp8.
  6. LF per query tile: rowsum (ones moving) + 2x512 DoubleRow matmuls
     per key-pair; DVE: rinv scale + SQ add; ACT tanh; DMA out.
"""

import numpy as np
import ml_dtypes

import concourse.bass as bass
import concourse.tile as tile
from concourse import bacc, mybir
from concourse import bass_utils
from concourse.masks import make_identity
from concourse.bass_interp import get_hw_module

F32 = mybir.dt.float32
F32R = mybir.dt.float32r
FP8 = mybir.dt.float8e4
ADD = mybir.AluOpType.add
MULT = mybir.AluOpType.mult
DR = mybir.MatmulPerfMode.DoubleRow
IDENT = mybir.ActivationFunctionType.Identity
EXP = mybir.ActivationFunctionType.Exp
TANH = mybir.ActivationFunctionType.Tanh

B, N, D = 4, 2048, 1024
NQ = N // 2          # queries per core
KT = D // 128        # 8 contraction tiles
MT = N // 128        # 16 key tiles
QT = NQ // 128       # 8 query tiles
NCH = N // 512       # 4 key chunks of 512
NPR = KT // 2        # 4 DoubleRow pairs over D
MPR = MT // 2        # 8 DoubleRow pairs over keys
WSCALE = 8.0         # host pre-scale on W_J / W_global for fp8 range
SCALE = 1.0 / np.sqrt(np.float32(D))

LAST_RESULT = None   # BassKernelResults of the most recent run (for test.py)
_CACHED = {}


def _build(debug=False, for_sim=False):
    nc = bacc.Bacc(
        "TRN2",
        target_bir_lowering=False,
        debug=False,
        enable_asserts=False,
        num_devices=8,
    )
    xt_d = nc.dram_tensor("xt", [128, KT, N], F32R, kind="ExternalInput").ap()
    wspin_d = nc.dram_tensor("wspinT", [128, KT, D], F32R, kind="ExternalInput").ap()
    wj_d = nc.dram_tensor("wj8", [128, KT, D], FP8, kind="ExternalInput").ap()
    wglob_d = nc.dram_tensor("wglob8", [128, KT, D], FP8, kind="ExternalInput").ap()
    bspin_d = nc.dram_tensor("bspin", [128, KT], F32, kind="ExternalInput").ap()
    beta_d = nc.dram_tensor("beta", [1, 1], F32, kind="ExternalInput").ap()
    out_d = nc.dram_tensor("out", [NQ, D], F32, kind="ExternalOutput").ap()

    with tile.TileContext(nc) as tc:
        with (
            tc.tile_pool(name="const", bufs=1) as const,
            tc.tile_pool(name="pbig", bufs=1) as pbig,
            tc.tile_pool(name="pst", bufs=1) as pst,
            tc.tile_pool(name="stats", bufs=8) as stats,
        ):
            ident32 = const.tile([128, 128], F32)
            make_identity(nc, ident32)
            ident_s = const.tile([128, 128], F32R)
            nc.vector.tensor_copy(ident_s[:], ident32[:])
            ones1f = const.tile([1, 128], F32)
            nc.vector.memset(ones1f, 1.0)
            ones1r = const.tile([1, 128], F32R)
            nc.vector.tensor_copy(ones1r[:], ones1f[:])
            ones2_f = const.tile([128, 2, 16], F32)
            nc.vector.memset(ones2_f, 1.0)
            ones2_8 = const.tile([128, 2, 16], FP8)
            nc.vector.tensor_copy(ones2_8[:], ones2_f[:])
            beta_sb = const.tile([128, 1], F32)
            nc.gpsimd.dma_start(out=beta_sb[:], in_=beta_d.to_broadcast((128, 1)))
            bspin_sb = const.tile([128, KT], F32)
            nc.gpsimd.dma_start(out=bspin_sb[:], in_=bspin_d[:])
            mf4 = const.tile([128, KT, NCH], F32)
            mf = const.tile([128, KT], F32)
            mf8 = const.tile([128, KT, 16], FP8)
            gT = const.tile([1, D], F32R)
            gfull = const.tile([128, D], F32)

            STQ = pbig.tile([128, KT, N], FP8)     # s fp8: [d-in-tile, d-tile, key]
            ST = pst.tile([128, KT, N], F32R)      # s^T f32r (freed w/ pst at end)
            wj_sb = pst.tile([128, KT, D], FP8)
            wglob_sb = pst.tile([128, KT, D], FP8)

            # ---- Phase 1: s^T = W_spin^T(kxo) . x^T(kxn) + bias; mf ----
            with tc.tile_pool(name="ph1", bufs=1) as ph1:
                wspin_sb = ph1.tile([128, KT, D], F32R)
                xtc = {}

                def load_chunk(nch):
                    t = ph1.tile([128, KT, 512], F32R, name=f"xtc{nch}", tag="xtc",
                                 bufs=2)
                    nc.sync.dma_start(
                        out=t[:], in_=xt_d[:, :, nch * 512:(nch + 1) * 512])
                    xtc[nch] = t

                def s_writeback(ot, sl, ps, nch):
                    # DVE: f32r ST + bias + mean-field chunk accum
                    nc.vector.tensor_scalar(
                        out=ST[:, ot, sl],
                        in0=ps[:],
                        scalar1=bspin_sb[:, ot:ot + 1],
                        scalar2=None,
                        op0=ADD, op1=ADD,
                        accum_out=mf4[:, ot, nch:nch + 1],
                    )
                    # GpSimd (otherwise idle): fp8 STQ cast from SBUF, so no
                    # extra PSUM reader slows the PE drain (+45ns/MM measured)
                    nc.gpsimd.tensor_copy(STQ[:, ot, sl], ST[:, ot, sl])

                # first pass needs only W_spin[kt 0:2] + chunk 0 (~3MB)
                nc.sync.dma_start(out=wspin_sb[:, 0:2, :], in_=wspin_d[:, 0:2, :])
                load_chunk(0)
                nc.sync.dma_start(out=wspin_sb[:, 2:4, :], in_=wspin_d[:, 2:4, :])
                nc.sync.dma_start(out=wspin_sb[:, 4:8, :], in_=wspin_d[:, 4:8, :])
                load_chunk(1)
                # fp8 weights ride the otherwise-idle ACT ring so they don't
                # queue behind the 12MB of x/W_spin input (the g matmuls
                # stalled ~8us on wglob8 when it was last on the SP ring)
                nc.scalar.dma_start(out=wj_sb[:], in_=wj_d[:])
                nc.scalar.dma_start(out=wglob_sb[:], in_=wglob_d[:])

                # chunk 0 in kt-split passes (N stays 512: narrower matmuls
                # pay unhidden LDWEIGHTS) so matmuls start at ~3MB of DMA
                with tc.tile_pool(name="ps1a", bufs=1, space="PSUM") as ps1a:
                    ps_n0 = [ps1a.tile([128, 512], F32, name=f"psn0_{ot}",
                                       tag=f"psn0_{ot}")
                             for ot in range(KT)]
                    kt0 = 0
                    for pi, klen in enumerate((2, 2, 4)):
                        for ot in range(KT):
                            for kt in range(kt0, kt0 + klen):
                                nc.tensor.matmul(
                                    ps_n0[ot][:],
                                    wspin_sb[:, kt, ot * 128:(ot + 1) * 128],
                                    xtc[0][:, kt, :],
                                    start=(kt == 0), stop=(kt == KT - 1),
                                )
                        kt0 += klen
                        if pi == 0:
                            # queue the remaining input DMA behind the hot ones
                            load_chunk(2)
                            load_chunk(3)
                    for ot in range(KT):
                        s_writeback(ot, slice(0, 512), ps_n0[ot], 0)

                with tc.tile_pool(name="ps1", bufs=6, space="PSUM") as ps1:
                    for nch in range(1, NCH):
                        sl = slice(nch * 512, (nch + 1) * 512)
                        xt_c = xtc[nch]
                        for ot in range(KT):
                            ps = ps1.tile([128, 512], F32)
                            for kt in range(KT):
                                nc.tensor.matmul(
                                    ps[:],
                                    wspin_sb[:, kt, ot * 128:(ot + 1) * 128],
                                    xt_c[:, kt, :],
                                    start=(kt == 0), stop=(kt == KT - 1),
                                )
                            s_writeback(ot, sl, ps, nch)
                for ot in range(KT):
                    nc.vector.reduce_sum(
                        out=mf[:, ot:ot + 1], in_=mf4[:, ot, :],
                        axis=mybir.AxisListType.X,
                    )
                # mf8 = mean * 32 (fp8 normal range); /32 folded into g copy
                nc.vector.tensor_scalar_mul(mf8[:, :, 0:1], mf[:], 32.0 / N)

            with tc.tile_pool(name="patt", bufs=1) as patt:
                EF = patt.tile([128, KT, NQ], FP8)   # 8*W_J*s: [d-in, d-tile, q]
                SN = patt.tile([128, MT, D], FP8)    # s+g: [key-in-tile, key-tile, d]
                SQ = patt.tile([128, QT, D], F32R)   # s: [q-in-tile, q-tile, d]
                PT = patt.tile([128, MT, NQ], FP8)   # exp(logits): [key-in, key-tile, q]

                # ---- Phase 2: EF^T = (8 W_J)^T . s^T (fp8 DoubleRow), with
                # the serial g chain (PE->DVE->PE->DVE) interleaved between
                # EF groups so the PE never idles and HAM stays warm ----
                with (
                    tc.tile_pool(name="ps2", bufs=3, space="PSUM") as ps2,
                    tc.tile_pool(name="psg", bufs=1, space="PSUM") as psg,
                    tc.tile_pool(name="psgf", bufs=1, space="PSUM") as psgf,
                ):
                    gps = psg.tile([1, 2, 512], F32)
                    gf_ps = psgf.tile([128, 2, 512], F32)

                    def g_stage(stage):
                        if stage == 0:
                            # gps = (mf*32/N) . (8 W_glob^T)
                            for ch in range(2):
                                for pr in range(NPR):
                                    nc.tensor.matmul(
                                        gps[:, ch, :],
                                        mf8[:, 2 * pr:2 * pr + 2, 0:1],
                                        wglob_sb[:, 2 * pr:2 * pr + 2,
                                                 ch * 512:(ch + 1) * 512],
                                        start=(pr == 0), stop=(pr == NPR - 1),
                                        perf_mode=DR,
                                    )
                        elif stage == 1:
                            nc.vector.tensor_scalar_mul(
                                gT[:],
                                gps[0:1, :, :].rearrange("p a b -> p (a b)"),
                                1.0 / (WSCALE * 32.0),
                            )
                        elif stage == 2:
                            # gfull = ones x gT (rank-1 broadcast)
                            for ch in range(2):
                                nc.tensor.matmul(
                                    gf_ps[:, ch, :], ones1r[:],
                                    gT[:, ch * 512:(ch + 1) * 512],
                                    start=True, stop=True,
                                )
                        elif stage == 3:
                            nc.vector.tensor_copy(
                                gfull[:], gf_ps[:].rearrange("p a b -> p (a b)"))

                    gi = 0
                    for ot in range(KT):
                        for ch in range(2):
                            ps = ps2.tile([128, 512], F32)
                            csl = slice(ch * 512, (ch + 1) * 512)
                            for pr in range(NPR):
                                nc.tensor.matmul(
                                    ps[:],
                                    wj_sb[:, 2 * pr:2 * pr + 2,
                                          ot * 128:(ot + 1) * 128],
                                    STQ[:, 2 * pr:2 * pr + 2, csl],
                                    start=(pr == 0), stop=(pr == NPR - 1),
                                    perf_mode=DR,
                                )
                            nc.vector.tensor_copy(EF[:, ot, csl], ps[:])
                        if ot in (1, 2, 3, 4) :
                            g_stage(gi)
                            gi += 1

                # ---- Phase 3+4 merged: scores^T per key tile (DoubleRow) +
                # exp -> PT, interleaved with the ST transposes (SN/SQ).
                # Transpose-mode doesn't count as PE-busy for HAM, so a
                # transpose-only stretch re-throttles the clock; the DR
                # matmuls in between keep it at K=8/8. ----
                with (
                    tc.tile_pool(name="ps4", bufs=2, space="PSUM") as ps4,
                    tc.tile_pool(name="ps3", bufs=4, space="PSUM") as ps3,
                ):
                    for mt in range(MT):
                        ps = ps4.tile([128, 2, 512], F32)
                        msl = slice(mt * 128, (mt + 1) * 128)
                        for pr in range(NPR):
                            for qch in range(2):
                                nc.tensor.matmul(
                                    ps[:, qch, :],
                                    STQ[:, 2 * pr:2 * pr + 2, msl],
                                    EF[:, 2 * pr:2 * pr + 2,
                                       qch * 512:(qch + 1) * 512],
                                    start=(pr == 0), stop=(pr == NPR - 1),
                                    perf_mode=DR,
                                )
                        for qch in range(2):
                            # no max subtraction: |logits| < ~2 here
                            nc.scalar.activation(
                                out=PT[:, mt, qch * 512:(qch + 1) * 512],
                                in_=ps[:, qch, :],
                                func=EXP, bias=0.0,
                                scale=float(SCALE / WSCALE),
                            )
                        for dq in range(KT // 4):
                            tp = ps3.tile([128, 4, 128], F32R)
                            for j in range(4):
                                nc.tensor.transpose(
                                    tp[:, j, :],
                                    ST[:, dq * 4 + j, msl],
                                    ident_s[:],
                                )
                            dsl4 = slice(dq * 512, (dq + 1) * 512)
                            nc.vector.tensor_add(
                                SN[:, mt, dsl4], tp[:], gfull[:, dsl4])
                            if mt < QT:
                                nc.scalar.copy(SQ[:, mt, dsl4], tp[:])

                # ---- Phase 5: LF + rowsum per query tile; normalize; out ----
                with (
                    tc.tile_pool(name="psr", bufs=2, space="PSUM") as psr,
                    tc.tile_pool(name="psl", bufs=2, space="PSUM") as psl,
                    tc.tile_pool(name="work", bufs=2) as work,
                ):
                    for qt in range(QT):
                        q0 = qt * 128
                        ps_r = psr.tile([128, 1], F32)
                        ps_l = psl.tile([128, 2, 512], F32)
                        for pr in range(MPR):
                            pT = PT[:, 2 * pr:2 * pr + 2, q0:q0 + 128]
                            nc.tensor.matmul(
                                ps_r[:], pT, ones2_8[:, :, 0:1],
                                start=(pr == 0), stop=(pr == MPR - 1),
                                perf_mode=DR,
                            )
                            for dch in range(2):
                                nc.tensor.matmul(
                                    ps_l[:, dch, :], pT,
                                    SN[:, 2 * pr:2 * pr + 2,
                                       dch * 512:(dch + 1) * 512],
                                    start=(pr == 0), stop=(pr == MPR - 1),
                                    perf_mode=DR,
                                )
                        rinv = stats.tile([128, 1], F32)
                        nc.vector.reciprocal(rinv[:], ps_r[:])
                        for dch in range(2):
                            dsl = slice(dch * 512, (dch + 1) * 512)
                            z = work.tile([128, 512], F32)
                            nc.vector.tensor_scalar_mul(
                                z[:], ps_l[:, dch, :], rinv[:])
                            z2 = work.tile([128, 512], F32)
                            nc.vector.tensor_add(z2[:], z[:], SQ[:, qt, dsl])
                            osb = work.tile([128, 512], F32, name="osb",
                                            tag="osb", bufs=4)
                            nc.scalar.activation(
                                out=osb[:], in_=z2[:],
                                func=TANH, bias=0.0, scale=beta_sb[:],
                            )
                            # alternate the two HW DGE rings (SP / ACT): a
                            # single ring serializes the 16x256KB stores into
                            # a ~12us tail after the last tanh
                            eng = nc.sync if dch == 0 else nc.scalar
                            eng.dma_start(
                                out=out_d[q0:q0 + 128, dsl], in_=osb[:])

    nc.compile()
    if not for_sim:
        nc.m = get_hw_module(nc.m)
    return nc


def _tile_kxm(a, np_dt):
    """(K, M) row-major -> [128, K//128, M] with k = kt*128 + p."""
    k, m = a.shape
    return np.ascontiguousarray(
        a.reshape(k // 128, 128, m).transpose(1, 0, 2)
    ).astype(np_dt)


def make_in_maps(x, W_spin, b_spin, W_global, W_J, beta):
    x = np.asarray(x, dtype=np.float32)
    W_spin = np.asarray(W_spin, dtype=np.float32)
    b_spin = np.asarray(b_spin, dtype=np.float32)
    W_global = np.asarray(W_global, dtype=np.float32)
    W_J = np.asarray(W_J, dtype=np.float32)
    beta = np.asarray(beta, dtype=np.float32)

    wspinT = _tile_kxm(W_spin.T, np.float32)   # W_spin.T is (k, o)
    wj8 = _tile_kxm(W_J.T * WSCALE, ml_dtypes.float8_e4m3)
    wglob8 = _tile_kxm(W_global.T * WSCALE, ml_dtypes.float8_e4m3)
    bspin = np.ascontiguousarray(b_spin.reshape(KT, 128).T).astype(np.float32)
    beta_h = beta.reshape(1, 1).astype(np.float32)

    in_maps = []
    for core in range(8):
        b, h = divmod(core, 2)
        xb = x[b]
        if h == 0:
            x_perm = xb
        else:
            x_perm = np.concatenate([xb[NQ:], xb[:NQ]], axis=0)
        xt = _tile_kxm(np.ascontiguousarray(x_perm.T), np.float32)  # (k=D, n=N)
        in_maps.append({
            "xt": xt, "wspinT": wspinT, "wj8": wj8, "wglob8": wglob8,
            "bspin": bspin, "beta": beta_h,
        })
    return in_maps


def kernel(x, W_spin, b_spin, W_global, W_J, beta):
    global LAST_RESULT
    if "hw" not in _CACHED:
        _CACHED["hw"] = _build()
    nc = _CACHED["hw"]

    in_maps = make_in_maps(x, W_spin, b_spin, W_global, W_J, beta)

    LAST_RESULT = bass_utils.run_bass_kernel_spmd(
        nc, in_maps, core_ids=list(range(8))
    )

    out = np.empty((B, N, D), dtype=np.float32)
    for core in range(8):
        b, h = divmod(core, 2)
        out[b, h * NQ:(h + 1) * NQ, :] = LAST_RESULT.results[core]["out"]
    return out
